# revision 15
# baseline (speedup 1.0000x reference)
"""Trainium2 Bass kernel for sliding-window unfold (im2col).

reference:  out = x[:, idx, :]  with idx[w, f] = w + f
  x:   [128, 4096, 4]  f32
  out: [128, 4065, 32, 4]  f32

out[b, w] (= 128 floats = 512 B) is the contiguous slice
x[b].flat[4w : 4w + 128]; the problem is a sliding-window byte
replication and HBM write bandwidth is the roofline.  Per core
(16 batches): 33.3 MB of output writes.  A deep SWDGE store queue
sustains ~420-435 GB/s; floor = ~78 us of store drain + ~10 us fixed
framework pre/post-amble + ~4 us ramp.

Hard-won scheduling facts (from NTFF traces of prior iterations):
  - DGE descriptor generation is serial at ~5-10 ns/desc; descriptor
    size = the final contiguous AP dim.  512 B-chunk DMAs top out near
    ~100 GB/s; big descriptors are everything.
  - HWDGE *stores* persistently degrade SDMA engine 15 to ~21 GB/s
    (vs 26.5) while other engines are active, unbalancing the drain by
    ~18 us.  All stores go on GPSIMD/SWDGE; HWDGE carries loads only.
  - A store whose DRAM-side partition stride is NOT uniform (e.g. a
    3-dim dst mixing a batch jump with a slice stride) can break the
    descriptor spray: engines get unequal bytes and per-packet rate
    halves; worst case it degenerates to 4 B descriptors.  The proven
    store shape is dst [[row,128],[1,row]]-style with uniform stride.
  - Tile inserts WAW semaphores between DMAs with overlapping DRAM
    ranges - keep all output writes strictly disjoint.
  - A DVE copy that enters 2-port perf mode locks GPSIMD off the
    shared SBUF port and stalls SWDGE descriptor emission.
  - Every dma_start costs ~0.6 us of trigger time on its issuing
    engine; completions fire ~2 us after the last byte.

Layout (per core):
  bulk: partition p holds windows 31p..31p+30 of one batch b.
    load X (248 f32/partition/batch), expand on ACT/DVE into
    Y[128, 3968] via an overlapping-stride read AP, store Y ->
    out[b] windows 0..3967 (contiguous 15.5 KB per partition ->
    128 fat descriptors at 26.5 GB/s/engine).  Batch 0's expand is
    split in half across DVE and ACT so the first store issues ~8.5 us.
  tail: windows 3968..4063 (disjoint from bulk): partition p = 8b+s
    holds 12 consecutive windows of batch b's tail (6 KB descriptors),
    expanded from a tiny raw load.  Window 4064 is contiguous x data:
    a [16, 128] tile (partition = batch) rides a single-engine
    load+store of 512 B descriptors, issued first to prime the pipe.
"""

import numpy as np

from concourse import bacc, mybir, tile
from concourse.bass_utils import run_bass_kernel_spmd

N_CORES = 8
B_FULL = 128
B = B_FULL // N_CORES  # 16 batches per core
S = 4096
C = 4
F = 32
W = S - F + 1    # 4065
FL = F * C       # 128 floats per window
XB = S * C       # 16384 floats per batch of x
OB = W * FL      # 520320 floats per batch of out
WPP = 31         # windows per partition in the bulk store
NBULK = 128 * WPP          # 3968 bulk windows per batch
YROW = WPP * FL            # 3968 floats per partition row
XROW = (WPP - 1) * C + FL  # 248 floats of x per partition per batch

# tail geometry: windows 3968..4063 as 8 slices of 12 windows per batch
# (partition p = 8*b + s, strictly disjoint writes); window 4064 is a
# [16, 128] raw load+store (partition = batch, contiguous 512 B rows).
TSL = 8                    # slices per batch
TWIN = 12                  # windows per slice
TSTR = 12                  # window stride between slices
TROW = TWIN * FL           # 1536 floats of tail output per partition
RLD = 176                  # floats of raw x loaded per partition
W4 = W - 1                 # window 4064
H0 = 8                     # windows in bulk batch-0 first piece (small
                           # so the first store issues ~7.6 us)
H1 = WPP - H0              # 23 windows in second piece (ACT)

_cache = {}


def build_nc():
    nc = bacc.Bacc("TRN2", target_bir_lowering=False)
    x = nc.dram_tensor("x", [B, S, C], mybir.dt.float32, kind="ExternalInput")
    out = nc.dram_tensor("out", [B, W, F, C], mybir.dt.float32, kind="ExternalOutput")
    scratch = nc.dram_tensor("scratch", [64], mybir.dt.float32, kind="Internal")

    with tile.TileContext(nc) as tc:
        with (
            tc.tile_pool(name="x01", bufs=2) as x01,
            tc.tile_pool(name="xg1", bufs=1) as xg1p,
            tc.tile_pool(name="xg2", bufs=1) as xg2p,
            tc.tile_pool(name="y0a", bufs=1) as y0ap,
            tc.tile_pool(name="y0b", bufs=1) as y0bp,
            tc.tile_pool(name="yp", bufs=10) as yp,
        ):
            def ld(engine, dst_tile, dst_ap, dst_off, src_ap, src_off):
                src = x[:].copy()
                src.ap = mybir.VecI64Pair(src_ap)
                src.offset = src_off
                dst = dst_tile[:].copy()
                dst.ap = mybir.VecI64Pair(dst_ap)
                dst.offset = dst_off
                engine.dma_start(out=dst, in_=src)

            def st(engine, src_tile, src_ap, src_off, dst_ap, dst_off):
                dst = out[:].copy()
                dst.ap = mybir.VecI64Pair(dst_ap)
                dst.offset = dst_off
                src = src_tile[:].copy()
                src.ap = mybir.VecI64Pair(src_ap)
                src.offset = src_off
                engine.dma_start(out=dst, in_=src)

            def expand(engine, src_tile, src_row, src_off, dst_tile, dst_row, nwin):
                src = src_tile[:].copy()
                src.ap = mybir.VecI64Pair([[src_row, 128], [C, nwin], [1, FL]])
                src.offset = src_off
                dst = dst_tile[:].copy()
                dst.ap = mybir.VecI64Pair([[dst_row, 128], [FL, nwin], [1, FL]])
                dst.offset = 0
                if engine is nc.vector:
                    engine.tensor_copy(out=dst, in_=src)
                else:
                    engine.copy(out=dst, in_=src)

            # ---- loads ----
            # sync ring: batch-0 head slice (just the H0 windows the first
            # expand needs, so the first store triggers ~1.5us earlier),
            # then batch 0, batch 1, batches 2..7
            XR0 = (H0 - 1) * C + FL  # 156 floats
            X0a = x01.tile([128, XR0], mybir.dt.float32)
            ld(nc.sync, X0a, [[XR0, 128], [1, XR0]], 0,
               [[WPP * C, 128], [1, XR0]], 0)
            X0 = x01.tile([128, XROW], mybir.dt.float32)
            ld(nc.sync, X0, [[XROW, 128], [1, XROW]], 0,
               [[WPP * C, 128], [1, XROW]], 0)
            X1 = x01.tile([128, XROW], mybir.dt.float32)
            ld(nc.sync, X1, [[XROW, 128], [1, XROW]], 0,
               [[WPP * C, 128], [1, XROW]], XB)
            XG1 = xg1p.tile([128, 6 * XROW], mybir.dt.float32)
            ld(nc.sync, XG1, [[6 * XROW, 128], [XROW, 6], [1, XROW]], 0,
               [[WPP * C, 128], [XB, 6], [1, XROW]], 2 * XB)
            # scalar ring: batches 8..15 (tail + window 4064 now go straight
            # from DRAM x to DRAM out as D2D stores - no loads needed)
            XG2 = xg2p.tile([128, 8 * XROW], mybir.dt.float32)
            ld(nc.scalar, XG2, [[8 * XROW, 128], [XROW, 8], [1, XROW]], 0,
               [[WPP * C, 128], [XB, 8], [1, XROW]], 8 * XB)

            # ---- expands ----
            # DVE: batch-0 first half, then odd batches (a 2-port-mode
            # DVE copy locks GPSIMD out of the shared SBUF port, so keep
            # DVE's queue clear while the first stores are emitted).
            Y0a = y0ap.tile([128, H0 * FL], mybir.dt.float32)
            expand(nc.vector, X0a, XR0, 0, Y0a, H0 * FL, H0)
            # ACT: batch-0 second half, tail expand, then even batches
            Y0b = y0bp.tile([128, H1 * FL], mybir.dt.float32)
            expand(nc.scalar, X0, XROW, H0 * C, Y0b, H1 * FL, H1)

            Ys = {}
            for b in range(1, B):
                eng = nc.vector if b % 2 == 1 else nc.scalar
                if b == 1:
                    src_t, row, off = X1, XROW, 0
                elif b < 8:
                    src_t, row, off = XG1, 6 * XROW, (b - 2) * XROW
                else:
                    src_t, row, off = XG2, 8 * XROW, (b - 8) * XROW
                Y = yp.tile([128, YROW], mybir.dt.float32)
                expand(eng, src_t, row, off, Y, YROW, WPP)
                Ys[b] = Y

            # ---- stores: ALL on GPSIMD/SWDGE ----
            # 16-desc 4B DRAM->DRAM warmup (x -> scratch, zero deps,
            # first in the queue): primes the SWDGE descriptor-fetch
            # pipe ~4us before the first real store, cutting its
            # trigger->first-packet latency from ~3.2us to ~1.3us.
            # 16 descriptors keep the engine ring aligned (D2D stores
            # carry no per-engine completion-semaphore descriptors).
            dst0 = scratch[:].copy()
            dst0.ap = mybir.VecI64Pair([[1, 16], [1, 1]])
            dst0.offset = 0
            src0 = x[:].copy()
            src0.ap = mybir.VecI64Pair([[1, 16], [1, 1]])
            src0.offset = 0
            nc.gpsimd.dma_start(out=dst0, in_=src0)
            # Tail windows 3968..4064 are contiguous 512 B slices of x, so
            # they go DRAM->DRAM with zero dependencies: their 1552 descs
            # are in the ring by ~9 us and drain during the window where
            # the engines would otherwise idle waiting on loads/expands
            # (~9-15 us), removing 0.76 MB from the late bulk drain.
            d2 = out[:].copy()
            d2.ap = mybir.VecI64Pair([[OB, B], [1, FL]])
            d2.offset = W4 * FL
            s2 = x[:].copy()
            s2.ap = mybir.VecI64Pair([[XB, B], [1, FL]])
            s2.offset = W4 * C
            nc.gpsimd.dma_start(out=d2, in_=s2)
            d3 = out[:].copy()
            d3.ap = mybir.VecI64Pair([[OB, B], [FL, W4 - NBULK], [1, FL]])
            d3.offset = NBULK * FL
            s3 = x[:].copy()
            s3.ap = mybir.VecI64Pair([[XB, B], [C, W4 - NBULK], [1, FL]])
            s3.offset = NBULK * C
            nc.gpsimd.dma_start(out=d3, in_=s3)
            # batch-0 head slice next: its dep chain (tiny X0a load ->
            # DVE expand) resolves earliest; the gpsimd queue is FIFO so
            # nothing with a later-resolving wait may sit ahead of it.
            st(nc.gpsimd, Y0a, [[H0 * FL, 128], [1, H0 * FL]], 0,
               [[YROW, 128], [1, H0 * FL]], 0)
            st(nc.gpsimd, Y0b, [[H1 * FL, 128], [1, H1 * FL]], 0,
               [[YROW, 128], [1, H1 * FL]], H0 * FL)
            for b in range(1, B):
                st(nc.gpsimd, Ys[b], [[YROW, 128], [1, YROW]], 0,
                   [[YROW, 128], [1, YROW]], b * OB)

    nc.finalize()
    return nc


def run_sharded(x: np.ndarray, trace: bool = False):
    """Shard batch across 8 cores, run, gather. Returns (out, raw results)."""
    if "nc" not in _cache:
        _cache["nc"] = build_nc()
    nc = _cache["nc"]

    x = np.ascontiguousarray(x, dtype=np.float32)
    in_maps = [{"x": x[i * B : (i + 1) * B]} for i in range(N_CORES)]
    res = run_bass_kernel_spmd(nc, in_maps, list(range(N_CORES)), trace=trace)
    out = np.concatenate([res.results[i]["out"] for i in range(N_CORES)], axis=0)
    return out, res


def kernel(x: np.ndarray) -> np.ndarray:
    out, _ = run_sharded(x, trace=False)
    return out



# revision 16
# speedup vs baseline: 1.0875x; 1.0875x over previous
"""Trainium2 Bass kernel for sliding-window unfold (im2col).

reference:  out = x[:, idx, :]  with idx[w, f] = w + f
  x:   [128, 4096, 4]  f32
  out: [128, 4065, 32, 4]  f32

out[b, w] (= 128 floats = 512 B) is the contiguous slice
x[b].flat[4w : 4w + 128]; the problem is a sliding-window byte
replication and HBM write bandwidth is the roofline.  Per core
(16 batches): 33.3 MB of output writes.  A deep SWDGE store queue
sustains ~420-435 GB/s; floor = ~78 us of store drain + ~10 us fixed
framework pre/post-amble + ~4 us ramp.

Hard-won scheduling facts (from NTFF traces of prior iterations):
  - DGE descriptor generation is serial at ~5-10 ns/desc; descriptor
    size = the final contiguous AP dim.  512 B-chunk DMAs top out near
    ~100 GB/s; big descriptors are everything.
  - HWDGE *stores* persistently degrade SDMA engine 15 to ~21 GB/s
    (vs 26.5) while other engines are active, unbalancing the drain by
    ~18 us.  All stores go on GPSIMD/SWDGE; HWDGE carries loads only.
  - A store whose DRAM-side partition stride is NOT uniform (e.g. a
    3-dim dst mixing a batch jump with a slice stride) can break the
    descriptor spray: engines get unequal bytes and per-packet rate
    halves; worst case it degenerates to 4 B descriptors.  The proven
    store shape is dst [[row,128],[1,row]]-style with uniform stride.
  - Tile inserts WAW semaphores between DMAs with overlapping DRAM
    ranges - keep all output writes strictly disjoint.
  - A DVE copy that enters 2-port perf mode locks GPSIMD off the
    shared SBUF port and stalls SWDGE descriptor emission.
  - Every dma_start costs ~0.6 us of trigger time on its issuing
    engine; completions fire ~2 us after the last byte.

Layout (per core):
  bulk: partition p holds windows 31p..31p+30 of one batch b.
    load X (248 f32/partition/batch), expand on ACT/DVE into
    Y[128, 3968] via an overlapping-stride read AP, store Y ->
    out[b] windows 0..3967 (contiguous 15.5 KB per partition ->
    128 fat descriptors at 26.5 GB/s/engine).  Batch 0's expand is
    split in half across DVE and ACT so the first store issues ~8.5 us.
  tail: windows 3968..4063 (disjoint from bulk): partition p = 8b+s
    holds 12 consecutive windows of batch b's tail (6 KB descriptors),
    expanded from a tiny raw load.  Window 4064 is contiguous x data:
    a [16, 128] tile (partition = batch) rides a single-engine
    load+store of 512 B descriptors, issued first to prime the pipe.
"""

import numpy as np

from concourse import bacc, mybir, tile
from concourse.bass_utils import run_bass_kernel_spmd

N_CORES = 8
B_FULL = 128
B = B_FULL // N_CORES  # 16 batches per core
S = 4096
C = 4
F = 32
W = S - F + 1    # 4065
FL = F * C       # 128 floats per window
XB = S * C       # 16384 floats per batch of x
OB = W * FL      # 520320 floats per batch of out
WPP = 31         # windows per partition in the bulk store
NBULK = 128 * WPP          # 3968 bulk windows per batch
YROW = WPP * FL            # 3968 floats per partition row
XROW = (WPP - 1) * C + FL  # 248 floats of x per partition per batch

# tail geometry: windows 3968..4063 as 8 slices of 12 windows per batch
# (partition p = 8*b + s, strictly disjoint writes); window 4064 is a
# [16, 128] raw load+store (partition = batch, contiguous 512 B rows).
TSL = 8                    # slices per batch
TWIN = 12                  # windows per slice
TSTR = 12                  # window stride between slices
TROW = TWIN * FL           # 1536 floats of tail output per partition
RLD = 176                  # floats of raw x loaded per partition
W4 = W - 1                 # window 4064
H0 = 8                     # windows in bulk batch-0 first piece (small
                           # so the first store issues ~7.6 us)
H1 = WPP - H0              # 23 windows in second piece (ACT)

_cache = {}


def build_nc():
    nc = bacc.Bacc("TRN2", target_bir_lowering=False)
    x = nc.dram_tensor("x", [B, S, C], mybir.dt.float32, kind="ExternalInput")
    out = nc.dram_tensor("out", [B, W, F, C], mybir.dt.float32, kind="ExternalOutput")

    with tile.TileContext(nc) as tc:
        with (
            tc.tile_pool(name="x01", bufs=2) as x01,
            tc.tile_pool(name="xg1", bufs=1) as xg1p,
            tc.tile_pool(name="xg2", bufs=1) as xg2p,
            tc.tile_pool(name="y0a", bufs=1) as y0ap,
            tc.tile_pool(name="y0b", bufs=1) as y0bp,
            tc.tile_pool(name="yp", bufs=10) as yp,
            tc.tile_pool(name="rp", bufs=1) as rp,
            tc.tile_pool(name="vp", bufs=1) as vp,
            tc.tile_pool(name="tp", bufs=1) as tp,
        ):
            def ld(engine, dst_tile, dst_ap, dst_off, src_ap, src_off):
                src = x[:].copy()
                src.ap = mybir.VecI64Pair(src_ap)
                src.offset = src_off
                dst = dst_tile[:].copy()
                dst.ap = mybir.VecI64Pair(dst_ap)
                dst.offset = dst_off
                engine.dma_start(out=dst, in_=src)

            def st(engine, src_tile, src_ap, src_off, dst_ap, dst_off):
                dst = out[:].copy()
                dst.ap = mybir.VecI64Pair(dst_ap)
                dst.offset = dst_off
                src = src_tile[:].copy()
                src.ap = mybir.VecI64Pair(src_ap)
                src.offset = src_off
                engine.dma_start(out=dst, in_=src)

            def expand(engine, src_tile, src_row, src_off, dst_tile, dst_row, nwin):
                src = src_tile[:].copy()
                src.ap = mybir.VecI64Pair([[src_row, 128], [C, nwin], [1, FL]])
                src.offset = src_off
                dst = dst_tile[:].copy()
                dst.ap = mybir.VecI64Pair([[dst_row, 128], [FL, nwin], [1, FL]])
                dst.offset = 0
                if engine is nc.vector:
                    engine.tensor_copy(out=dst, in_=src)
                else:
                    engine.copy(out=dst, in_=src)

            # ---- loads ----
            # sync ring: batch 0, batch 1, batches 2..7
            X0 = x01.tile([128, XROW], mybir.dt.float32)
            ld(nc.sync, X0, [[XROW, 128], [1, XROW]], 0,
               [[WPP * C, 128], [1, XROW]], 0)
            X1 = x01.tile([128, XROW], mybir.dt.float32)
            ld(nc.sync, X1, [[XROW, 128], [1, XROW]], 0,
               [[WPP * C, 128], [1, XROW]], XB)
            XG1 = xg1p.tile([128, 6 * XROW], mybir.dt.float32)
            ld(nc.sync, XG1, [[6 * XROW, 128], [XROW, 6], [1, XROW]], 0,
               [[WPP * C, 128], [XB, 6], [1, XROW]], 2 * XB)
            # scalar ring: window-4064 raw load (partition = batch), tail
            # raw load, then batches 8..15
            V = vp.tile([16, FL], mybir.dt.float32)
            ld(nc.scalar, V, [[FL, 16], [1, FL]], 0,
               [[XB, B], [1, FL]], W4 * C)
            R = rp.tile([128, RLD], mybir.dt.float32)
            ld(nc.scalar, R, [[RLD, 128], [1, RLD]], 0,
               [[XB, B], [TSTR * C, TSL], [1, RLD]], NBULK * C)
            XG2 = xg2p.tile([128, 8 * XROW], mybir.dt.float32)
            ld(nc.scalar, XG2, [[8 * XROW, 128], [XROW, 8], [1, XROW]], 0,
               [[WPP * C, 128], [XB, 8], [1, XROW]], 8 * XB)

            # ---- expands ----
            # DVE: batch-0 first half, then odd batches (a 2-port-mode
            # DVE copy locks GPSIMD out of the shared SBUF port, so keep
            # DVE's queue clear while the first stores are emitted).
            Y0a = y0ap.tile([128, H0 * FL], mybir.dt.float32)
            expand(nc.vector, X0, XROW, 0, Y0a, H0 * FL, H0)
            # ACT: batch-0 second half, tail expand, then even batches
            Y0b = y0bp.tile([128, H1 * FL], mybir.dt.float32)
            expand(nc.scalar, X0, XROW, H0 * C, Y0b, H1 * FL, H1)
            T = tp.tile([128, TROW], mybir.dt.float32)
            expand(nc.scalar, R, RLD, 0, T, TROW, TWIN)

            Ys = {}
            for b in range(1, B):
                eng = nc.vector if b % 2 == 1 else nc.scalar
                if b == 1:
                    src_t, row, off = X1, XROW, 0
                elif b < 8:
                    src_t, row, off = XG1, 6 * XROW, (b - 2) * XROW
                else:
                    src_t, row, off = XG2, 8 * XROW, (b - 8) * XROW
                Y = yp.tile([128, YROW], mybir.dt.float32)
                expand(eng, src_t, row, off, Y, YROW, WPP)
                Ys[b] = Y

            # ---- stores: ALL on GPSIMD/SWDGE ----
            # window-4064 first (no expand dependency, primes the pipe;
            # 16 partitions -> one engine, 64 KB, harmless early).
            st(nc.gpsimd, V, [[FL, 16], [1, FL]], 0,
               [[OB, B], [1, FL]], W4 * FL)
            # batch-0 halves, then the tail, then batches 1..15; the
            # queue ends on clean uniform 15.5 KB-descriptor stores.
            st(nc.gpsimd, Y0a, [[H0 * FL, 128], [1, H0 * FL]], 0,
               [[YROW, 128], [1, H0 * FL]], 0)
            st(nc.gpsimd, Y0b, [[H1 * FL, 128], [1, H1 * FL]], 0,
               [[YROW, 128], [1, H1 * FL]], H0 * FL)
            st(nc.gpsimd, T, [[TROW, 128], [1, TROW]], 0,
               [[OB, B], [TSTR * FL, TSL], [1, TROW]], NBULK * FL)
            for b in range(1, B):
                st(nc.gpsimd, Ys[b], [[YROW, 128], [1, YROW]], 0,
                   [[YROW, 128], [1, YROW]], b * OB)

    nc.finalize()
    return nc


def run_sharded(x: np.ndarray, trace: bool = False):
    """Shard batch across 8 cores, run, gather. Returns (out, raw results)."""
    if "nc" not in _cache:
        _cache["nc"] = build_nc()
    nc = _cache["nc"]

    x = np.ascontiguousarray(x, dtype=np.float32)
    in_maps = [{"x": x[i * B : (i + 1) * B]} for i in range(N_CORES)]
    res = run_bass_kernel_spmd(nc, in_maps, list(range(N_CORES)), trace=trace)
    out = np.concatenate([res.results[i]["out"] for i in range(N_CORES)], axis=0)
    return out, res


def kernel(x: np.ndarray) -> np.ndarray:
    out, _ = run_sharded(x, trace=False)
    return out

